# revision 1
# baseline (speedup 1.0000x reference)
"""Trainium2 Bass kernel for ConvBnSign (binarized 3x3 conv + sync-BN + sign).

Math: y = conv2d(x, sign(w) * alpha)  with alpha = mean|w| per out-channel,
then train-mode BatchNorm over (N,H,W), then hard_sign.

Since alpha_o > 0 is a per-channel scale, fold it into the BN affine:
  z = conv2d(x, sign(w))          (exact +-1 weights -> exact in bf16)
  y = alpha * z; mean_y = alpha*mu_z; var_y = alpha^2*var_z
  out = sign((z - mu_z) * A + beta)  with A = alpha*gamma*rsqrt(alpha^2 var_z + eps)
      = sign(z*A + B),  B = beta - mu_z*A

Precision: x is split on host into bf16 hi + lo (combined ~2^-18 relative);
each 3x3 tap is two accumulating bf16 matmuls into fp32 PSUM.

Sharding: data-parallel, 4 images per core across 8 cores; BN stats are
per-channel partial sums [128,4] fp32 all-reduced across cores.
"""

import numpy as np
import ml_dtypes

import concourse.bass as bass
import concourse.mybir as mybir
import concourse.tile as tile
from concourse.vector_clock import ScopedClock
from concourse.bass_utils import run_bass_kernel_spmd

# ---- problem constants (hardcoded per contract) ----
N_CORES = 8
N_FULL = 32           # batch
CIN = 128             # input channels
COUT = 256            # output channels
H = W = 56
KH = KW = 3
BN_EPS = 1e-5

IMGS = N_FULL // N_CORES          # 4 images per core
WP = W + 2                        # 58 padded width
HP = H + 2
PADPIX = HP * WP                  # 3364
PIX = H * W                       # 3136
NCHUNK = COUT // 128              # 2 chunks of 128 output channels
RTR = 8                           # rows per matmul tile
RT = H // RTR                     # 7 row tiles per image
NTILE = RTR * W                   # 448 = matmul free dim (<=512, one PSUM bank)
NTOT = N_FULL * PIX               # 200704 elements per channel for BN stats

BF16 = mybir.dt.bfloat16
F32 = mybir.dt.float32

_MAX_DRAIN_WAITS = 1  # walrus CTRL instructions accept a single sync wait


def _split_multi_waits(nc, max_waits=1):
    """This walrus build rejects instructions with more than one sem wait.
    Hoist excess waits onto same-engine NoOps inserted immediately before the
    offending instruction (the engine blocks at the NoOp instead — identical
    ordering semantics)."""
    ctr = 0
    for bbw in nc.main_func.blocks:
        out = []
        changed = False
        for inst in bbw.instructions:
            si = inst.sync_info
            w = list(si.on_wait or []) if si else []
            if len(w) > max_waits:
                changed = True
                excess = w[: len(w) - max_waits]
                for i in range(0, len(excess), max_waits):
                    nop = mybir.InstNoOp(name=f"WFIX-{ctr}", ins=[], outs=[])
                    ctr += 1
                    nop.engine = inst.engine
                    nop.sync_info = mybir.SyncInfo(
                        on_wait=excess[i : i + max_waits], on_update=[]
                    )
                    out.append(nop)
                inst.sync_info = mybir.SyncInfo(
                    on_wait=w[len(w) - max_waits :],
                    on_update=list(si.on_update or []),
                )
            out.append(inst)
        if changed:
            bbw.instructions = out
    return ctr


class _SplitDrainTileContext(tile.TileContext):
    """TileContext whose final drain splits its sem waits across multiple
    sync-engine instructions (this walrus build caps CTRL waits at 1)."""

    def _drain_and_barrier(self, tick_clock, wait_clock):
        drain_inst = self.nc.sync.drain()
        wait_clock.add_sem_waits(
            drain_inst.ins, ScopedClock({None: tick_clock.global_clock})
        )
        si = drain_inst.ins.sync_info
        w = list(si.on_wait or [])
        if len(w) > _MAX_DRAIN_WAITS:
            drain_inst.ins.sync_info = mybir.SyncInfo(
                on_wait=w[:_MAX_DRAIN_WAITS], on_update=list(si.on_update or [])
            )
            for i in range(_MAX_DRAIN_WAITS, len(w), _MAX_DRAIN_WAITS):
                nop = self.nc.sync.nop(nofuse=True)
                nop.ins.sync_info = mybir.SyncInfo(
                    on_wait=w[i : i + _MAX_DRAIN_WAITS], on_update=[]
                )
        self.nc.all_engine_barrier()
        assert self.sems is not None
        popped = self.nc._tile_sem_poison_stack.pop()
        assert popped is self._sem_poison
        self.nc.clear_and_free_semaphores(list(self.sems.allocated().values()))
        self.nc.all_engine_barrier()


def build_bass(n_cores=N_CORES, collective=True):
    """Build the per-core Bass module (SPMD: same program on every core)."""
    nc = bass.Bass(num_devices=n_cores)

    xh_d = nc.dram_tensor("xh", [IMGS, CIN, PADPIX], BF16, kind="ExternalInput")
    xl_d = nc.dram_tensor("xl", [IMGS, CIN, PADPIX], BF16, kind="ExternalInput")
    ws_d = nc.dram_tensor("ws", [CIN, KH * KW * COUT], BF16, kind="ExternalInput")
    abg_d = nc.dram_tensor("abg", [128, 3 * NCHUNK], F32, kind="ExternalInput")
    out_d = nc.dram_tensor("out", [IMGS, NCHUNK, 128, PIX], BF16,
                           kind="ExternalOutput")

    with _SplitDrainTileContext(nc) as tc:
        with (
            tc.tile_pool(name="const", bufs=1) as constp,
            tc.tile_pool(name="xbuf", bufs=1) as xp,
            tc.tile_pool(name="zbuf", bufs=1) as zp,
            tc.tile_pool(name="stats", bufs=1) as sp,
            tc.tile_pool(name="sq", bufs=2) as sqp,
            tc.tile_pool(name="pz", bufs=8, space="PSUM") as pp,
            tc.tile_pool(name="dram", bufs=1, space="DRAM") as dp,
        ):
            # ---- constants ----
            w_sb = constp.tile([128, KH * KW * COUT], BF16, tag="wsgn")
            abg_sb = constp.tile([128, 3 * NCHUNK], F32, tag="abg")
            nc.sync.dma_start(w_sb[:], ws_d[:])
            nc.sync.dma_start(abg_sb[:], abg_d[:])
            w_v = w_sb[:].rearrange("p (k o) -> p k o", k=KH * KW)

            # ---- x tiles (per image, hi/lo) ----
            xt = {}
            for img in range(IMGS):
                for half, src in (("h", xh_d), ("l", xl_d)):
                    t = xp.tile([128, PADPIX], BF16, tag=f"x{half}{img}", name=f"x{half}{img}")
                    nc.sync.dma_start(t[:], src[img])
                    xt[(half, img)] = t

            # ---- z buffers + stats ----
            z = [zp.tile([128, IMGS * PIX], F32, tag=f"z{j}", name=f"z{j}")
                 for j in range(NCHUNK)]
            ssum = sp.tile([128, 64], F32, tag="ssum")
            ssq = sp.tile([128, 64], F32, tag="ssq")

            alpha = abg_sb[:, 0:NCHUNK]
            gamma = abg_sb[:, NCHUNK : 2 * NCHUNK]
            beta = abg_sb[:, 2 * NCHUNK : 3 * NCHUNK]
            inv_n = 1.0 / NTOT
            npart = IMGS * RT

            # hi taps first: PE can start before any x_lo has arrived
            taps = [(k, "h") for k in range(KH * KW)] + \
                   [(k, "l") for k in range(KH * KW)]

            # Per chunk: conv -> stats AllReduce -> sign+store. Chunk 0's
            # collective + BN tail overlaps chunk 1's conv on PE.
            for j in range(NCHUNK):
                for img in range(IMGS):
                    # tile-major: one PSUM tile completes every 18 matmuls, so
                    # bank releases interleave smoothly with the next tile's
                    # compute (ldw-opt is off, so weight-major saved nothing)
                    for rt in range(RT):
                        pt = pp.tile([128, NTILE], F32, tag="pz",
                                     name=f"pz{j}_{img}_{rt}")
                        for widx, (k, half) in enumerate(taps):
                            dy, dx = divmod(k, KW)
                            lhsT = w_v[:, k, j * 128 : (j + 1) * 128]
                            xv = xt[(half, img)][:].rearrange(
                                "p (r c) -> p r c", r=HP
                            )
                            rhs = xv[:, rt * RTR + dy : rt * RTR + dy + RTR,
                                     dx : dx + W]
                            nc.tensor.matmul(
                                pt[:], lhsT, rhs,
                                start=(widx == 0), stop=(widx == len(taps) - 1),
                            )
                        col = img * RT + rt
                        zs = z[j][:, img * PIX + rt * NTILE
                                  : img * PIX + (rt + 1) * NTILE]
                        nc.vector.tensor_scalar(
                            out=zs, in0=pt[:], scalar1=0.0, scalar2=None,
                            op0=mybir.AluOpType.add, op1=mybir.AluOpType.add,
                            accum_out=ssum[:, j * npart + col
                                           : j * npart + col + 1],
                        )
                        sqt = sqp.tile([128, NTILE], F32, tag="sqt")
                        nc.scalar.activation(
                            out=sqt[:], in_=pt[:],
                            func=mybir.ActivationFunctionType.Square,
                            accum_out=ssq[:, j * npart + col
                                          : j * npart + col + 1],
                        )

                # ---- chunk-j stats: [128,2] = (sum, sumsq) ----
                cc_sb = sp.tile([128, 2], F32, tag=f"ccsb{j}", name=f"ccsb{j}")
                nc.vector.reduce_sum(
                    out=cc_sb[:, 0:1], in_=ssum[:, j * npart : (j + 1) * npart],
                    axis=mybir.AxisListType.X,
                )
                nc.vector.reduce_sum(
                    out=cc_sb[:, 1:2], in_=ssq[:, j * npart : (j + 1) * npart],
                    axis=mybir.AxisListType.X,
                )
                st = sp.tile([128, 2], F32, tag=f"st{j}", name=f"st{j}")
                if collective and n_cores > 1:
                    cc_in = dp.tile([128, 2], F32, tag=f"ccin{j}",
                                    name=f"ccin{j}")
                    cc_out = dp.tile([128, 2], F32, tag=f"ccout{j}",
                                     name=f"ccout{j}")
                    nc.sync.dma_start(cc_in[:], cc_sb[:])
                    nc.gpsimd.collective_compute(
                        "AllReduce", mybir.AluOpType.add,
                        replica_groups=[list(range(n_cores))],
                        ins=[cc_in.opt()], outs=[cc_out.opt()],
                    )
                    nc.sync.dma_start(st[:], cc_out[:])
                else:
                    nc.vector.tensor_copy(st[:], cc_sb[:])

                # ---- A, B for chunk j:  out = sign(z*A + B) ----
                al, ga, be = (v[:, j : j + 1] for v in (alpha, gamma, beta))
                mu = sp.tile([128, 1], F32, tag=f"mu{j}", name=f"mu{j}")
                var = sp.tile([128, 1], F32, tag=f"var{j}", name=f"var{j}")
                A = sp.tile([128, 1], F32, tag=f"A{j}", name=f"A{j}")
                B = sp.tile([128, 1], F32, tag=f"B{j}", name=f"B{j}")
                tmp = sp.tile([128, 1], F32, tag=f"tmp{j}", name=f"tmp{j}")

                nc.scalar.mul(mu[:], st[:, 0:1], inv_n)          # mu = s/n
                nc.scalar.mul(var[:], st[:, 1:2], inv_n)         # E[z^2]
                nc.vector.tensor_tensor(out=tmp[:], in0=mu[:], in1=mu[:],
                                        op=mybir.AluOpType.mult)
                nc.vector.tensor_tensor(out=var[:], in0=var[:], in1=tmp[:],
                                        op=mybir.AluOpType.subtract)
                nc.vector.tensor_tensor(out=tmp[:], in0=al, in1=al,
                                        op=mybir.AluOpType.mult)
                nc.vector.tensor_tensor(out=var[:], in0=var[:], in1=tmp[:],
                                        op=mybir.AluOpType.mult)
                nc.vector.tensor_scalar(out=var[:], in0=var[:],
                                        scalar1=float(BN_EPS), scalar2=None,
                                        op0=mybir.AluOpType.add)
                nc.scalar.sqrt(var[:], var[:])
                nc.vector.reciprocal(var[:], var[:])     # rsqrt(a^2 var + eps)
                nc.vector.tensor_tensor(out=tmp[:], in0=al, in1=ga,
                                        op=mybir.AluOpType.mult)
                nc.vector.tensor_tensor(out=A[:], in0=tmp[:], in1=var[:],
                                        op=mybir.AluOpType.mult)
                nc.vector.tensor_tensor(out=tmp[:], in0=mu[:], in1=A[:],
                                        op=mybir.AluOpType.mult)
                nc.vector.tensor_tensor(out=B[:], in0=be, in1=tmp[:],
                                        op=mybir.AluOpType.subtract)

                # ---- sign(z*A + B) -> bf16 staging -> DRAM ----
                for img in range(IMGS):
                    ostg = sqp.tile([128, PIX], BF16, tag="ostg",
                                    name=f"ostg{j}_{img}")
                    nc.scalar.activation(
                        out=ostg[:], in_=z[j][:, img * PIX : (img + 1) * PIX],
                        func=mybir.ActivationFunctionType.Sign,
                        bias=B[:, 0:1], scale=A[:, 0:1],
                    )
                    nc.sync.dma_start(out_d[img, j], ostg[:])

    _split_multi_waits(nc)
    return nc


def _prep_inputs(x, weight, gamma, beta):
    """Host-side prep: alpha/sign folding, padding, bf16 hi/lo split."""
    x = np.ascontiguousarray(x, dtype=np.float32)
    weight = np.ascontiguousarray(weight, dtype=np.float32)

    alpha = np.abs(weight).mean(axis=(1, 2, 3)).astype(np.float32)      # [256]
    sgn = np.where(weight >= 0, np.float32(1), np.float32(-1))          # [256,128,3,3]
    # ws[cin, k*256 + o] = sgn[o, cin, dy, dx],  k = dy*3+dx
    ws = np.ascontiguousarray(
        sgn.transpose(1, 2, 3, 0).reshape(CIN, KH * KW * COUT)
    ).astype(ml_dtypes.bfloat16)

    # abg[p, j] layout: [alpha(2) | gamma(2) | beta(2)], channel o = j*128+p
    def chunked(v):
        return np.ascontiguousarray(v.reshape(NCHUNK, 128).T)  # [128, 2]
    abg = np.concatenate(
        [chunked(alpha), chunked(np.asarray(gamma, np.float32)),
         chunked(np.asarray(beta, np.float32))], axis=1
    ).astype(np.float32)                                                # [128, 6]

    xpad = np.zeros((N_FULL, CIN, HP, WP), np.float32)
    xpad[:, :, 1 : H + 1, 1 : W + 1] = x
    xh = xpad.astype(ml_dtypes.bfloat16)
    xl = (xpad - xh.astype(np.float32)).astype(ml_dtypes.bfloat16)
    xh = xh.reshape(N_FULL, CIN, PADPIX)
    xl = xl.reshape(N_FULL, CIN, PADPIX)

    in_maps = []
    for c in range(N_CORES):
        sl = slice(c * IMGS, (c + 1) * IMGS)
        in_maps.append({
            "xh": np.ascontiguousarray(xh[sl]),
            "xl": np.ascontiguousarray(xl[sl]),
            "ws": ws,
            "abg": abg,
        })
    return in_maps


def kernel(x, weight, gamma, beta):
    in_maps = _prep_inputs(x, weight, gamma, beta)
    nc = build_bass()
    res = run_bass_kernel_spmd(nc, in_maps, core_ids=list(range(N_CORES)))
    out = np.empty((N_FULL, COUT, H, W), np.float32)
    for c in range(N_CORES):
        o = res.results[c]["out"]          # [IMGS, 2, 128, 3136] bf16 (+-1)
        o = o.astype(np.float32).reshape(IMGS, COUT, H, W)
        out[c * IMGS : (c + 1) * IMGS] = o
    return out



# revision 2
# speedup vs baseline: 1.8011x; 1.8011x over previous
"""Trainium2 Bass kernel for ConvBnSign (binarized 3x3 conv + sync-BN + sign).

Math: y = conv2d(x, sign(w) * alpha)  with alpha = mean|w| per out-channel,
then train-mode BatchNorm over (N,H,W), then hard_sign.

Since alpha_o > 0 is a per-channel scale, fold it into the BN affine:
  z = conv2d(x, sign(w))          (exact +-1 weights)
  out = sign((z - mu_z) * A + beta)  with A = alpha*gamma*rsqrt(alpha^2 var_z + eps)
      = sign(z*A + B),  B = beta - mu_z*A

Precision: x is split on host into 4 e4m3 terms (term p stores
e4m3(residual * 2^(4p)); combined residual ~2^-18 relative). Weights are
sign(w) * 2^(-4p) in e5m2 (exact powers of two). All 4 terms accumulate
into one fp32 PSUM group, so one conv pass carries full precision.

Speed: fp8 DoubleRow matmuls contract 2 k-subtiles (2x128) per
instruction at 0.5 cycles/row -- 4x bf16 MAC throughput. The 9 taps x 4
terms = 36 virtual taps pack into 18 DoubleRow matmuls per PSUM tile
(custom strided APs pair taps whose padded-image offsets differ by a
constant), i.e. half the PE cycles of a single bf16 hi/lo scheme.

Sharding: data-parallel, 4 images per core across 8 cores; BN stats are
per-channel partial sums [128,4] fp32 all-reduced across cores.
"""

import numpy as np
import ml_dtypes

import concourse.bass as bass
import concourse.mybir as mybir
import concourse.tile as tile
from concourse.vector_clock import ScopedClock
from concourse.bass_utils import run_bass_kernel_spmd

# ---- problem constants (hardcoded per contract) ----
N_CORES = 8
N_FULL = 32           # batch
CIN = 128             # input channels
COUT = 256            # output channels
H = W = 56
KH = KW = 3
BN_EPS = 1e-5

IMGS = N_FULL // N_CORES          # 4 images per core
WP = W + 2                        # 58 padded width
HP = H + 2
PADPIX = HP * WP                  # 3364
PIX = H * W                       # 3136
NCHUNK = COUT // 128              # 2 chunks of 128 output channels
RTR = 8                           # rows per matmul tile
RT = H // RTR                     # 7 row tiles per image
NTILE = RTR * W                   # 448 = matmul free dim (<=512, one PSUM bank)
NTOT = N_FULL * PIX               # 200704 elements per channel for BN stats

NPASS = 4                         # e4m3 residual terms
TAPS = [(dy, dx) for dy in range(KH) for dx in range(KW)]
TAP_OFF = [dy * WP + dx for dy, dx in TAPS]
PAIRS = [(0, 1), (2, 3), (4, 5), (6, 7)]   # within-pass tap pairs
NMM = NPASS * len(PAIRS) + NPASS // 2      # 18 DoubleRow matmuls per tile

BF16 = mybir.dt.bfloat16
F32 = mybir.dt.float32
E4 = mybir.dt.float8e4
E5 = mybir.dt.float8e5

_MAX_DRAIN_WAITS = 1  # walrus CTRL instructions accept a single sync wait


def _split_multi_waits(nc, max_waits=1):
    """This walrus build rejects instructions with more than one sem wait.
    Hoist excess waits onto same-engine NoOps inserted immediately before the
    offending instruction (the engine blocks at the NoOp instead — identical
    ordering semantics)."""
    ctr = 0
    for bbw in nc.main_func.blocks:
        out = []
        changed = False
        for inst in bbw.instructions:
            si = inst.sync_info
            w = list(si.on_wait or []) if si else []
            if len(w) > max_waits:
                changed = True
                excess = w[: len(w) - max_waits]
                for i in range(0, len(excess), max_waits):
                    nop = mybir.InstNoOp(name=f"WFIX-{ctr}", ins=[], outs=[])
                    ctr += 1
                    nop.engine = inst.engine
                    nop.sync_info = mybir.SyncInfo(
                        on_wait=excess[i : i + max_waits], on_update=[]
                    )
                    out.append(nop)
                inst.sync_info = mybir.SyncInfo(
                    on_wait=w[len(w) - max_waits :],
                    on_update=list(si.on_update or []),
                )
            out.append(inst)
        if changed:
            bbw.instructions = out
    return ctr


class _SplitDrainTileContext(tile.TileContext):
    """TileContext whose final drain splits its sem waits across multiple
    sync-engine instructions (this walrus build caps CTRL waits at 1)."""

    def _drain_and_barrier(self, tick_clock, wait_clock):
        drain_inst = self.nc.sync.drain()
        wait_clock.add_sem_waits(
            drain_inst.ins, ScopedClock({None: tick_clock.global_clock})
        )
        si = drain_inst.ins.sync_info
        w = list(si.on_wait or [])
        if len(w) > _MAX_DRAIN_WAITS:
            drain_inst.ins.sync_info = mybir.SyncInfo(
                on_wait=w[:_MAX_DRAIN_WAITS], on_update=list(si.on_update or [])
            )
            for i in range(_MAX_DRAIN_WAITS, len(w), _MAX_DRAIN_WAITS):
                nop = self.nc.sync.nop(nofuse=True)
                nop.ins.sync_info = mybir.SyncInfo(
                    on_wait=w[i : i + _MAX_DRAIN_WAITS], on_update=[]
                )
        self.nc.all_engine_barrier()
        assert self.sems is not None
        popped = self.nc._tile_sem_poison_stack.pop()
        assert popped is self._sem_poison
        self.nc.clear_and_free_semaphores(list(self.sems.allocated().values()))
        self.nc.all_engine_barrier()


def build_bass(n_cores=N_CORES, collective=True):
    """Build the per-core Bass module (SPMD: same program on every core)."""
    nc = bass.Bass(num_devices=n_cores)

    xq_d = nc.dram_tensor("xq", [IMGS, NPASS, CIN, PADPIX], E4,
                          kind="ExternalInput")
    ws_d = nc.dram_tensor("ws", [CIN, NCHUNK, NMM, 2, 128], E5,
                          kind="ExternalInput")
    abg_d = nc.dram_tensor("abg", [128, 3 * NCHUNK], F32, kind="ExternalInput")
    out_d = nc.dram_tensor("out", [IMGS, NCHUNK, 128, PIX], E4,
                           kind="ExternalOutput")

    with _SplitDrainTileContext(nc) as tc:
        with (
            tc.tile_pool(name="const", bufs=1) as constp,
            tc.tile_pool(name="xbuf", bufs=1) as xp,
            tc.tile_pool(name="zbuf", bufs=1) as zp,
            tc.tile_pool(name="stats", bufs=1) as sp,
            tc.tile_pool(name="sq", bufs=2) as sqp,
            tc.tile_pool(name="pz", bufs=8, space="PSUM") as pp,
            tc.tile_pool(name="dram", bufs=1, space="DRAM") as dp,
        ):
            # ---- constants ----
            w_sb = constp.tile([128, NCHUNK, NMM, 2, 128], E5, tag="wsgn")
            abg_sb = constp.tile([128, 3 * NCHUNK], F32, tag="abg")
            for j in range(NCHUNK):
                nc.sync.dma_start(w_sb[:, j], ws_d[:, j])
            nc.sync.dma_start(abg_sb[:], abg_d[:])

            # ---- x tiles (per image, 4 fp8 terms each) ----
            xt = []
            for img in range(IMGS):
                t = xp.tile([128, NPASS, PADPIX], E4, tag=f"x{img}",
                            name=f"x{img}")
                for p in range(NPASS):
                    nc.sync.dma_start(t[:, p], xq_d[img, p])
                xt.append(t)

            # ---- z buffers + stats ----
            z = [zp.tile([128, IMGS * PIX], F32, tag=f"z{j}", name=f"z{j}")
                 for j in range(NCHUNK)]
            ssum = sp.tile([128, 64], F32, tag="ssum")
            ssq = sp.tile([128, 64], F32, tag="ssq")

            alpha = abg_sb[:, 0:NCHUNK]
            gamma = abg_sb[:, NCHUNK : 2 * NCHUNK]
            beta = abg_sb[:, 2 * NCHUNK : 3 * NCHUNK]
            inv_n = 1.0 / NTOT
            npart = IMGS * RT

            # Per chunk: conv -> stats AllReduce -> sign+store. Chunk 0's
            # collective + BN tail overlaps chunk 1's conv on PE.
            for j in range(NCHUNK):
                for img in range(IMGS):
                    xflat = xt[img][:].rearrange("p a f -> p (a f)")

                    def rhs_view(base, jstride):
                        c = xflat[:, base : base + 1].copy()
                        c.ap.pop()
                        c.ap.append((jstride, 2))
                        c.ap.append((WP, RTR))
                        c.ap.append((1, W))
                        return c

                    for rt in range(RT):
                        row0 = rt * RTR * WP
                        pt = pp.tile([128, NTILE], F32, tag="pz",
                                     name=f"pz{j}_{img}_{rt}")
                        mm = 0
                        for p in range(NPASS):
                            for (ta, tb) in PAIRS:
                                rhs = rhs_view(
                                    p * PADPIX + row0 + TAP_OFF[ta],
                                    TAP_OFF[tb] - TAP_OFF[ta],
                                )
                                nc.tensor.matmul(
                                    pt[:], w_sb[:, j, mm], rhs,
                                    start=(mm == 0), stop=False,
                                    perf_mode=mybir.MatmulPerfMode.DoubleRow,
                                )
                                mm += 1
                        for p in (0, 2):
                            rhs = rhs_view(
                                p * PADPIX + row0 + TAP_OFF[8], PADPIX
                            )
                            nc.tensor.matmul(
                                pt[:], w_sb[:, j, mm], rhs,
                                start=False, stop=(mm == NMM - 1),
                                perf_mode=mybir.MatmulPerfMode.DoubleRow,
                            )
                            mm += 1

                        col = img * RT + rt
                        zs = z[j][:, img * PIX + rt * NTILE
                                  : img * PIX + (rt + 1) * NTILE]
                        nc.vector.tensor_scalar(
                            out=zs, in0=pt[:], scalar1=0.0, scalar2=None,
                            op0=mybir.AluOpType.add, op1=mybir.AluOpType.add,
                            accum_out=ssum[:, j * npart + col
                                           : j * npart + col + 1],
                        )
                        sqt = sqp.tile([128, NTILE], F32, tag="sqt")
                        nc.scalar.activation(
                            out=sqt[:], in_=pt[:],
                            func=mybir.ActivationFunctionType.Square,
                            accum_out=ssq[:, j * npart + col
                                          : j * npart + col + 1],
                        )

                # ---- chunk-j stats: [128,2] = (sum, sumsq) ----
                cc_sb = sp.tile([128, 2], F32, tag=f"ccsb{j}", name=f"ccsb{j}")
                nc.vector.reduce_sum(
                    out=cc_sb[:, 0:1], in_=ssum[:, j * npart : (j + 1) * npart],
                    axis=mybir.AxisListType.X,
                )
                nc.vector.reduce_sum(
                    out=cc_sb[:, 1:2], in_=ssq[:, j * npart : (j + 1) * npart],
                    axis=mybir.AxisListType.X,
                )
                st = sp.tile([128, 2], F32, tag=f"st{j}", name=f"st{j}")
                if collective and n_cores > 1:
                    cc_in = dp.tile([128, 2], F32, tag=f"ccin{j}",
                                    name=f"ccin{j}")
                    cc_out = dp.tile([128, 2], F32, tag=f"ccout{j}",
                                     name=f"ccout{j}")
                    nc.sync.dma_start(cc_in[:], cc_sb[:])
                    nc.gpsimd.collective_compute(
                        "AllReduce", mybir.AluOpType.add,
                        replica_groups=[list(range(n_cores))],
                        ins=[cc_in.opt()], outs=[cc_out.opt()],
                    )
                    nc.sync.dma_start(st[:], cc_out[:])
                else:
                    nc.vector.tensor_copy(st[:], cc_sb[:])

                # ---- A, B for chunk j:  out = sign(z*A + B) ----
                al, ga, be = (v[:, j : j + 1] for v in (alpha, gamma, beta))
                mu = sp.tile([128, 1], F32, tag=f"mu{j}", name=f"mu{j}")
                var = sp.tile([128, 1], F32, tag=f"var{j}", name=f"var{j}")
                A = sp.tile([128, 1], F32, tag=f"A{j}", name=f"A{j}")
                B = sp.tile([128, 1], F32, tag=f"B{j}", name=f"B{j}")
                tmp = sp.tile([128, 1], F32, tag=f"tmp{j}", name=f"tmp{j}")

                nc.scalar.mul(mu[:], st[:, 0:1], inv_n)          # mu = s/n
                nc.scalar.mul(var[:], st[:, 1:2], inv_n)         # E[z^2]
                nc.vector.tensor_tensor(out=tmp[:], in0=mu[:], in1=mu[:],
                                        op=mybir.AluOpType.mult)
                nc.vector.tensor_tensor(out=var[:], in0=var[:], in1=tmp[:],
                                        op=mybir.AluOpType.subtract)
                nc.vector.tensor_tensor(out=tmp[:], in0=al, in1=al,
                                        op=mybir.AluOpType.mult)
                nc.vector.tensor_tensor(out=var[:], in0=var[:], in1=tmp[:],
                                        op=mybir.AluOpType.mult)
                nc.vector.tensor_scalar(out=var[:], in0=var[:],
                                        scalar1=float(BN_EPS), scalar2=None,
                                        op0=mybir.AluOpType.add)
                nc.scalar.sqrt(var[:], var[:])
                nc.vector.reciprocal(var[:], var[:])     # rsqrt(a^2 var + eps)
                nc.vector.tensor_tensor(out=tmp[:], in0=al, in1=ga,
                                        op=mybir.AluOpType.mult)
                nc.vector.tensor_tensor(out=A[:], in0=tmp[:], in1=var[:],
                                        op=mybir.AluOpType.mult)
                nc.vector.tensor_tensor(out=tmp[:], in0=mu[:], in1=A[:],
                                        op=mybir.AluOpType.mult)
                nc.vector.tensor_tensor(out=B[:], in0=be, in1=tmp[:],
                                        op=mybir.AluOpType.subtract)

                # ---- sign(z*A + B) -> fp8 staging -> DRAM ----
                for img in range(IMGS):
                    ostg = sqp.tile([128, PIX], E4, tag="ostg",
                                    name=f"ostg{j}_{img}")
                    nc.scalar.activation(
                        out=ostg[:], in_=z[j][:, img * PIX : (img + 1) * PIX],
                        func=mybir.ActivationFunctionType.Sign,
                        bias=B[:, 0:1], scale=A[:, 0:1],
                    )
                    nc.sync.dma_start(out_d[img, j], ostg[:])

    _split_multi_waits(nc)
    return nc


def _prep_inputs(x, weight, gamma, beta):
    """Host-side prep: alpha/sign folding, padding, 4-term e4m3 split."""
    x = np.ascontiguousarray(x, dtype=np.float32)
    weight = np.ascontiguousarray(weight, dtype=np.float32)

    alpha = np.abs(weight).mean(axis=(1, 2, 3)).astype(np.float32)      # [256]
    sgn = np.where(weight >= 0, np.float32(1), np.float32(-1))          # [256,128,3,3]
    # sgn_t[cin, dy, dx, o]
    sgn_t = np.ascontiguousarray(sgn.transpose(1, 2, 3, 0))             # [128,3,3,256]
    sgn_t = sgn_t.reshape(CIN, KH * KW, NCHUNK, 128)                    # tap-major

    # weights per DoubleRow matmul: [cin, chunk, mm, 2, 128] with term-p
    # scale 2^(-4p) folded in (exact in e5m2)
    wq = np.zeros((CIN, NCHUNK, NMM, 2, 128), np.float32)
    mm = 0
    for p in range(NPASS):
        s = 2.0 ** (-4 * p)
        for (ta, tb) in PAIRS:
            wq[:, :, mm, 0] = sgn_t[:, ta] * s
            wq[:, :, mm, 1] = sgn_t[:, tb] * s
            mm += 1
    for p in (0, 2):
        wq[:, :, mm, 0] = sgn_t[:, 8] * (2.0 ** (-4 * p))
        wq[:, :, mm, 1] = sgn_t[:, 8] * (2.0 ** (-4 * (p + 1)))
        mm += 1
    ws = wq.astype(ml_dtypes.float8_e5m2)

    # abg[p, j] layout: [alpha(2) | gamma(2) | beta(2)], channel o = j*128+p
    def chunked(v):
        return np.ascontiguousarray(v.reshape(NCHUNK, 128).T)  # [128, 2]
    abg = np.concatenate(
        [chunked(alpha), chunked(np.asarray(gamma, np.float32)),
         chunked(np.asarray(beta, np.float32))], axis=1
    ).astype(np.float32)                                                # [128, 6]

    # 4-term e4m3 residual split of padded x
    xpad = np.zeros((N_FULL, CIN, HP, WP), np.float32)
    xpad[:, :, 1 : H + 1, 1 : W + 1] = x
    terms = []
    r = xpad
    for p in range(NPASS):
        v = (r * np.float32(2.0 ** (4 * p))).astype(ml_dtypes.float8_e4m3fn)
        terms.append(v.reshape(N_FULL, CIN, PADPIX))
        if p < NPASS - 1:
            r = r - v.astype(np.float32) * np.float32(2.0 ** (-4 * p))
    xq = np.stack(terms, axis=1)          # [N, NPASS, CIN, PADPIX] e4m3

    in_maps = []
    for c in range(N_CORES):
        sl = slice(c * IMGS, (c + 1) * IMGS)
        in_maps.append({
            "xq": np.ascontiguousarray(xq[sl]),
            "ws": ws,
            "abg": abg,
        })
    return in_maps


def kernel(x, weight, gamma, beta):
    in_maps = _prep_inputs(x, weight, gamma, beta)
    nc = build_bass()
    res = run_bass_kernel_spmd(nc, in_maps, core_ids=list(range(N_CORES)))
    out = np.empty((N_FULL, COUT, H, W), np.float32)
    for c in range(N_CORES):
        o = res.results[c]["out"]          # [IMGS, 2, 128, 3136] e4m3 (+-1)
        o = o.astype(np.float32).reshape(IMGS, COUT, H, W)
        out[c * IMGS : (c + 1) * IMGS] = o
    return out
